# revision 37
# baseline (speedup 1.0000x reference)
"""Trainium2 Bass kernel for a single attention head.

Problem: X[4,4096,1024], Wq/Wk/Wv[1024,128] ->
  softmax((X@Wq)(X@Wk)^T / sqrt(1024)) @ (X@Wv)   -> [4,4096,128]

Sharding: 8 cores = 4 batches x 2 query-halves. The host hands each core
X^T (bf16, [1024, 4096]) of its batch, rolled so the core's query half is
columns [0:2048) — a pure layout/dtype transform; all FLOPs (projections,
scores, softmax, output) run on device.

On-core pipeline (matmuls bf16 in, fp32 PSUM):
  - X^T is loaded with plain contiguous HWDGE DMAs, column-chunk-major so
    projections start after the first chunk (~3us).
  - Projections per 512-key chunk: K^T[h,n], V^T[h,k-chunk] (-> V[k,h] via
    a small XBAR transpose), Q^T[h,q] for chunks 0-3. PSUM->SBUF copies on
    the ACT engine.
  - Transposed flash attention, software-pipelined so the PE never stalls
    on the exp: emit S(kt), exp(kt), O(kt-1). Projection chunks are
    interleaved into the attention stream to keep the PE dense.
  - Softmax denominator: one DVE chain accumulating pT; l via ones-matmul;
    1/l via reciprocal_approx_fast; broadcast to [128,q] by a rank-1 ones
    matmul; one DVE multiply. The q0 epilogue is staged across q1
    iterations so the PE queue never blocks on DVE results.
  - O^T [h, q] is DMA'd out and transposed on the host.
"""

import numpy as np
import ml_dtypes

BF16 = np.dtype(ml_dtypes.bfloat16)

B, N, D, H = 4, 4096, 1024, 128
NCORES = 8
QSPLIT = 2  # cores per batch (query halves)
NQ = N // QSPLIT
SCALE = 1.0 / float(np.sqrt(np.float32(D)))
P = 128   # partitions
FB = 512  # matmul free-dim block (one fp32 PSUM bank)
CR = 512  # keys per projection chunk
QC = 1024  # query chunk (attention)


def emit_attention(tc, XT, Wq, Wk, Wv, O):
    """Emit the single-core attention program into TileContext tc.

    XT: [NC, P, DT*CR] bf16 DRAM — X^T packed partition-major per key
    chunk: XT[c, p, t*CR+nb] = X[c*CR+nb, t*P+p] (queries are columns
    [0:NQ) of X^T). W*: [D, H] bf16; O: [H, NQ] f32 (transposed output).
    """
    import concourse.mybir as mybir

    nc = tc.nc
    dt = mybir.dt
    f32, bf16 = dt.float32, dt.bfloat16
    AF = mybir.ActivationFunctionType

    DT = D // P        # 8 contraction tiles for projections
    NT = N // P        # 32 key tiles
    NC = N // CR       # 8 key chunks
    NTC = CR // P      # 4 key tiles per chunk
    QCH = NQ // QC     # 2 query chunks

    from contextlib import ExitStack

    with ExitStack() as ctx:
        cpool = ctx.enter_context(tc.tile_pool(name="const", bufs=1))
        big = ctx.enter_context(tc.tile_pool(name="big", bufs=1))
        vtp = ctx.enter_context(tc.tile_pool(name="vt", bufs=4))
        ptp = ctx.enter_context(tc.tile_pool(name="pt", bufs=6))
        accd = ctx.enter_context(tc.tile_pool(name="accd", bufs=2))
        epp = ctx.enter_context(tc.tile_pool(name="ep", bufs=2))
        # PSUM: pp(2 banks) + stp(4) + acco q0(2) = 8; pp closes before
        # the q1 acco pool opens (phase B: stp 4 + acco0 2 + acco1 2).
        stp = ctx.enter_context(tc.tile_pool(name="stps", bufs=2, space="PSUM"))

        ones_col = cpool.tile([P, 1], f32, tag="onesc")
        nc.gpsimd.memset(ones_col[:], 1.0)
        ones_row = cpool.tile([1, P], f32, tag="onesr")
        nc.gpsimd.memset(ones_row[:], 1.0)
        ones_bf = cpool.tile([P, 1], bf16, tag="onesb")
        nc.gpsimd.memset(ones_bf[:], 1.0)
        ones_rb = cpool.tile([1, P], bf16, tag="onesrb")
        nc.gpsimd.memset(ones_rb[:], 1.0)

        # weights arrive pre-packed partition-major: W[p, t*H+h].
        # wk rides first on sync (the first projection needs it); wv/wq
        # go behind chunk 0's scalar half, in first-use order.
        w_sb = {}
        for name, w in (("wk", Wk), ("wv", Wv), ("wq", Wq)):
            w_sb[name] = cpool.tile([P, DT * H], bf16, tag=name, name=name)
        nc.gpsimd.dma_start(w_sb["wk"][:], Wk)

        kT = big.tile([P, N], bf16, tag="kT")          # K^T[h, n]
        qT = big.tile([P, NQ], bf16, tag="qT")         # Q^T[h, q]
        v_sb = big.tile([P, NT * H], bf16, tag="vsb")  # V[k%128, (kt, h)]
        # X^T resident in SBUF, chunk-major: xt[p, ((c, t), nb)]
        xt = big.tile([P, DT * N], bf16, tag="xt")
        xt4 = xt[:].rearrange("p (c t nb) -> p c t nb", c=NC, t=DT)

        # one contiguous 8KB-per-partition load per key chunk; chunk 0
        # split by partition halves across both HWDGE queues to land
        # first, later chunks alternate queues for aggregate bandwidth
        nc.sync.dma_start(xt4[0:64, 0], XT[0][0:64, :])
        nc.scalar.dma_start(xt4[64:128, 0], XT[0][64:128, :])
        nc.scalar.dma_start(w_sb["wv"][:], Wv)
        nc.scalar.dma_start(w_sb["wq"][:], Wq)
        # chunks 3 and 7 (most slack) ride the otherwise-idle Pool
        # SWDGE queue; the rest alternate the two fast HWDGE queues
        qmap = {1: nc.sync, 2: nc.scalar, 3: nc.gpsimd, 4: nc.sync,
                5: nc.scalar, 6: nc.sync, 7: nc.gpsimd}
        for c in range(1, NC):
            qmap[c].dma_start(xt4[:, c], XT[c])

        v_sb3 = v_sb[:].rearrange("p (kt h) -> p kt h", h=H)
        vxpose_todo = []  # (chunk, vT tile) awaiting transpose dispatch

        def flush_vxpose():
            while vxpose_todo:
                pc, vt = vxpose_todo.pop()
                nc.scalar.dma_start_transpose(
                    v_sb3[:, pc * NTC:(pc + 1) * NTC, :], vt[:]
                )

        def project(pp, wname, dst, c):
            """One projection for chunk c: 8 matmuls + one copy.

            K/Q copies go to DVE, V copies to ACT (keeps the vt ->
            vxpose ordering natural on the ACT queue)."""
            ps = pp.tile([P, CR], f32, tag="pps", name="pps")
            for t in range(DT):
                nc.tensor.matmul(
                    ps[:],
                    w_sb[wname][:, t * H:(t + 1) * H],
                    xt4[:, c, t],
                    start=(t == 0),
                    stop=(t == DT - 1),
                )
            if wname == "wv":
                nc.scalar.copy(dst, ps[:])
            else:
                nc.vector.tensor_copy(dst, ps[:])

        def proj_block(pp, c):
            flush_vxpose()
            project(pp, "wk", kT[:, c * CR:(c + 1) * CR], c)
            vt = vtp.tile([P, CR], bf16, tag="vt", name="vt")
            project(pp, "wv", vt[:], c)
            vxpose_todo.append((c, vt))
            if c * CR < NQ:
                project(pp, "wq", qT[:, c * CR:(c + 1) * CR], c)

        # ---- attention state ----
        acco = [None] * QCH    # PSUM accumulator tiles, one per q-chunk
        acc_d = [None] * QCH   # f32 chain heads (updated every 4th kt)
        tree = {}              # stashed pT / pair tiles for the bf16 tree
        pending = [None]       # (q_idx, kt, pT) with O-matmul not yet emitted

        def emit_O(q_idx, kt, pT):
            out_ps = acco[q_idx]
            for j in range(0, QC, FB):
                nc.tensor.matmul(
                    out_ps[:, j:j + FB],
                    v_sb[:, kt * H:(kt + 1) * H],
                    pT[:, j:j + FB],
                    start=(kt == 0), stop=(kt == NT - 1),
                )

        def attn_iter(q_idx, kt):
            q0 = q_idx * QC
            st = stp.tile([P, QC], f32, tag="st", name="st")
            for j in range(0, QC, FB):
                nc.tensor.matmul(
                    st[:, j:j + FB],
                    kT[:, kt * P:(kt + 1) * P],
                    qT[:, q0 + j:q0 + j + FB],
                    start=True, stop=True,
                )
            pT = ptp.tile([P, QC], bf16, tag="pt", name="pT")
            nc.scalar.activation(pT[:], st[:], AF.Exp, scale=SCALE)
            if pending[0] is not None:
                emit_O(*pending[0])
            pending[0] = (q_idx, kt, pT)
            if kt % 2 == 0:
                tree["pT"] = pT
            else:
                pair = accd.tile([P, QC], bf16, tag="pair", name="pair")
                nc.vector.tensor_add(pair[:], tree.pop("pT")[:], pT[:])
                if kt % 4 == 1:
                    tree["pair"] = pair
                else:
                    quad = accd.tile([P, QC], bf16, tag="quad", name="quad")
                    nc.vector.tensor_add(quad[:], tree.pop("pair")[:], pair[:])
                    if kt == NT - 1:
                        # last quad goes straight into the l-matmul (PE)
                        # instead of the serial f32 chain on the tail
                        tree[("lq", q_idx)] = quad
                    else:
                        # the kt==27 add rounds the head to bf16: fp32
                        # matmuls run at 4 cyc/row, bf16 at 1; the l
                        # error from rounding is ~0.4%/sqrt(128)
                        odt = bf16 if kt == NT - 5 else f32
                        nacc = accd.tile(
                            [P, QC], odt, tag=f"acc{q_idx}{odt}", name="nacc"
                        )
                        if kt == 3:
                            nc.vector.tensor_copy(nacc[:], quad[:])
                        else:
                            nc.vector.tensor_add(
                                nacc[:], acc_d[q_idx][:], quad[:]
                            )
                        acc_d[q_idx] = nacc

        # Epilogue, staged so each PE op's DVE input is long since ready.
        epi = {}

        def epi_lmm(q_idx):
            l_ps = stp.tile([P, QC], f32, tag="st", name="lps")  # row 0 used
            acc_m = acc_d[q_idx]            # covers kt 0..27
            lq = tree.pop(("lq", q_idx))    # covers kt 28..31
            for j in range(0, QC, FB):
                nc.tensor.matmul(
                    l_ps[0:1, j:j + FB], ones_bf[:], acc_m[:, j:j + FB],
                    start=True, stop=False,
                )
                nc.tensor.matmul(
                    l_ps[0:1, j:j + FB], ones_bf[:], lq[:, j:j + FB],
                    start=False, stop=True,
                )
            epi[(q_idx, "l")] = l_ps

        def epi_recip(q_idx):
            r_sb = epp.tile([1, QC], f32, tag="rsb", name="rsb")
            for j in range(0, QC, FB):
                nc.vector.reciprocal_approx_fast(
                    out=r_sb[:, j:j + FB],
                    in_=epi[(q_idx, "l")][0:1, j:j + FB],
                )
            # bf16 copy (on the idle Pool engine) so the broadcast matmul
            # runs at 1 cycle/row instead of fp32's 4
            r_bf = epp.tile([1, QC], bf16, tag="rbf", name="rbf")
            nc.gpsimd.tensor_copy(r_bf[:], r_sb[:])
            epi[(q_idx, "r")] = r_bf

        def epi_rbmm(q_idx):
            rb_ps = stp.tile([P, QC], f32, tag="st", name="rbps")
            for j in range(0, QC, FB):
                nc.tensor.matmul(
                    rb_ps[:, j:j + FB], ones_rb[:],
                    epi[(q_idx, "r")][:, j:j + FB],
                    start=True, stop=True,
                )
            epi[(q_idx, "rb")] = rb_ps

        def epi_out(q_idx):
            # at the q1 tail the ACT engine is idle: let it do the PSUM
            # copy so the DVE only runs the multiplies
            q0 = q_idx * QC
            rb_sb = epp.tile([P, QC], f32, tag="rbsb", name="rbsb")
            o_sb = epp.tile([P, QC], f32, tag="osb", name="osb")
            for j in range(0, QC, FB):
                if q_idx == 0:
                    nc.vector.tensor_copy(
                        rb_sb[:, j:j + FB], epi[(q_idx, "rb")][:, j:j + FB]
                    )
                else:
                    nc.scalar.copy(
                        rb_sb[:, j:j + FB], epi[(q_idx, "rb")][:, j:j + FB]
                    )
                nc.vector.tensor_mul(
                    o_sb[:, j:j + FB],
                    acco[q_idx][:, j:j + FB], rb_sb[:, j:j + FB],
                )
                nc.sync.dma_start(
                    O[:, q0 + j:q0 + j + FB], o_sb[:, j:j + FB]
                )

        # ---- interleaved emission ----
        with tc.tile_pool(name="acc0", bufs=1, space="PSUM") as a0:
            acco[0] = a0.tile([P, QC], f32, tag="out0", name="out0")
            with tc.tile_pool(name="pp", bufs=2, space="PSUM") as pp:
                proj_block(pp, 0)
                proj_block(pp, 1)
                for c in range(2, NC):
                    proj_block(pp, c)
                    for kt in range(NTC * (c - 2), NTC * (c - 1)):
                        attn_iter(0, kt)
            flush_vxpose()
            # phase B head: finish q-chunk 0 (keys from chunks 6-7)
            for kt in range(NTC * (NC - 2), NT):
                attn_iter(0, kt)
            # pp closed above -> banks free for q-chunk 1's accumulator
            with tc.tile_pool(name="acc1", bufs=1, space="PSUM") as a1:
                acco[1] = a1.tile([P, QC], f32, tag="out1", name="out1")
                for kt in range(NT):
                    attn_iter(1, kt)
                    if kt == 4:
                        epi_lmm(0)
                    elif kt == 6:
                        epi_recip(0)
                    elif kt == 8:
                        epi_rbmm(0)
                    elif kt == 10:
                        epi_out(0)
                emit_O(*pending[0])
                pending[0] = None
                epi_lmm(1)
                epi_recip(1)
                epi_rbmm(1)
                epi_out(1)


def build_bass():
    import concourse.mybir as mybir
    from concourse import bacc
    from concourse.tile import TileContext

    dt = mybir.dt
    nc = bacc.Bacc("TRN2", target_bir_lowering=False, debug=False)
    XT = nc.dram_tensor(
        "XT", [N // CR, P, (D // P) * CR], dt.bfloat16, kind="ExternalInput"
    ).ap()
    Wq = nc.dram_tensor(
        "Wq", [P, (D // P) * H], dt.bfloat16, kind="ExternalInput"
    ).ap()
    Wk = nc.dram_tensor(
        "Wk", [P, (D // P) * H], dt.bfloat16, kind="ExternalInput"
    ).ap()
    Wv = nc.dram_tensor(
        "Wv", [P, (D // P) * H], dt.bfloat16, kind="ExternalInput"
    ).ap()
    O = nc.dram_tensor("O", [H, NQ], dt.float32, kind="ExternalOutput").ap()

    with TileContext(nc) as tc:
        emit_attention(tc, XT, Wq, Wk, Wv, O)
    nc.compile()
    return nc


_CACHED = {}


def _get_nc():
    if "nc" not in _CACHED:
        _CACHED["nc"] = build_bass()
    return _CACHED["nc"]


def kernel(X, Wq, Wk, Wv, trace=False):
    """Full-input entry point: X [4,4096,1024] f32 -> [4,4096,128] f32."""
    from concourse.bass_utils import run_bass_kernel_spmd

    X = np.ascontiguousarray(X, dtype=np.float32)

    def pack_w(w):
        # W[d, h] -> W[p, t*H+h] with d = t*P + p (partition-major)
        w = np.asarray(w, dtype=np.float32).astype(BF16)
        return np.ascontiguousarray(
            w.reshape(D // P, P, H).transpose(1, 0, 2).reshape(P, -1)
        )

    wq, wk, wv = pack_w(Wq), pack_w(Wk), pack_w(Wv)

    nc = _get_nc()
    in_maps = []
    for core in range(NCORES):
        b, half = core // QSPLIT, core % QSPLIT
        # X^T (bf16) packed partition-major per key chunk, rolled so
        # this core's queries are columns [0:NQ) of X^T
        xb = X[b]
        if half:
            xb = np.concatenate([xb[NQ:], xb[:NQ]], axis=0)
        # XTB[c, p, t, nb] = X[c*CR+nb, t*P+p]
        xtb = np.ascontiguousarray(
            xb.reshape(N // CR, CR, D // P, P).transpose(0, 3, 2, 1)
            .astype(BF16)
            .reshape(N // CR, P, (D // P) * CR)
        )
        in_maps.append({"XT": xtb, "Wq": wq, "Wk": wk, "Wv": wv})

    res = run_bass_kernel_spmd(
        nc, in_maps, core_ids=list(range(NCORES)), trace=trace
    )
    out = np.empty((B, N, H), dtype=np.float32)
    for core in range(NCORES):
        b, half = core // QSPLIT, core % QSPLIT
        out[b, half * NQ:(half + 1) * NQ] = res.results[core]["O"].T
    if trace:
        return out, res
    return out


# revision 39
# speedup vs baseline: 1.0269x; 1.0269x over previous
"""Trainium2 Bass kernel for a single attention head.

Problem: X[4,4096,1024], Wq/Wk/Wv[1024,128] ->
  softmax((X@Wq)(X@Wk)^T / sqrt(1024)) @ (X@Wv)   -> [4,4096,128]

Sharding: 8 cores = 4 batches x 2 query-halves. The host hands each core
X^T (bf16, [1024, 4096]) of its batch, rolled so the core's query half is
columns [0:2048) — a pure layout/dtype transform; all FLOPs (projections,
scores, softmax, output) run on device.

On-core pipeline (matmuls bf16 in, fp32 PSUM):
  - X^T is loaded with plain contiguous HWDGE DMAs, column-chunk-major so
    projections start after the first chunk (~3us).
  - Projections per 512-key chunk: K^T[h,n], V^T[h,k-chunk] (-> V[k,h] via
    a small XBAR transpose), Q^T[h,q] for chunks 0-3. PSUM->SBUF copies on
    the ACT engine.
  - Transposed flash attention, software-pipelined so the PE never stalls
    on the exp: emit S(kt), exp(kt), O(kt-1). Projection chunks are
    interleaved into the attention stream to keep the PE dense.
  - Softmax denominator: one DVE chain accumulating pT; l via ones-matmul;
    1/l via reciprocal_approx_fast; broadcast to [128,q] by a rank-1 ones
    matmul; one DVE multiply. The q0 epilogue is staged across q1
    iterations so the PE queue never blocks on DVE results.
  - O^T [h, q] is DMA'd out and transposed on the host.
"""

import numpy as np
import ml_dtypes

BF16 = np.dtype(ml_dtypes.bfloat16)

B, N, D, H = 4, 4096, 1024, 128
NCORES = 8
QSPLIT = 2  # cores per batch (query halves)
NQ = N // QSPLIT
SCALE = 1.0 / float(np.sqrt(np.float32(D)))
P = 128   # partitions
FB = 512  # matmul free-dim block (one fp32 PSUM bank)
CR = 512  # keys per projection chunk
QC = 1024  # query chunk (attention)


def emit_attention(tc, XT, Wq, Wk, Wv, O):
    """Emit the single-core attention program into TileContext tc.

    XT: [NC, P, DT*CR] bf16 DRAM — X^T packed partition-major per key
    chunk: XT[c, p, t*CR+nb] = X[c*CR+nb, t*P+p] (queries are columns
    [0:NQ) of X^T). W*: [D, H] bf16; O: [H, NQ] f32 (transposed output).
    """
    import concourse.mybir as mybir

    nc = tc.nc
    dt = mybir.dt
    f32, bf16 = dt.float32, dt.bfloat16
    AF = mybir.ActivationFunctionType

    DT = D // P        # 8 contraction tiles for projections
    NT = N // P        # 32 key tiles
    NC = N // CR       # 8 key chunks
    NTC = CR // P      # 4 key tiles per chunk
    QCH = NQ // QC     # 2 query chunks

    from contextlib import ExitStack

    with ExitStack() as ctx:
        cpool = ctx.enter_context(tc.tile_pool(name="const", bufs=1))
        big = ctx.enter_context(tc.tile_pool(name="big", bufs=1))
        vtp = ctx.enter_context(tc.tile_pool(name="vt", bufs=4))
        ptp = ctx.enter_context(tc.tile_pool(name="pt", bufs=6))
        accd = ctx.enter_context(tc.tile_pool(name="accd", bufs=2))
        epp = ctx.enter_context(tc.tile_pool(name="ep", bufs=2))
        # PSUM: pp(2 banks) + stp(4) + acco q0(2) = 8; pp closes before
        # the q1 acco pool opens (phase B: stp 4 + acco0 2 + acco1 2).
        stp = ctx.enter_context(tc.tile_pool(name="stps", bufs=2, space="PSUM"))

        ones_col = cpool.tile([P, 1], f32, tag="onesc")
        nc.gpsimd.memset(ones_col[:], 1.0)
        ones_row = cpool.tile([1, P], f32, tag="onesr")
        nc.gpsimd.memset(ones_row[:], 1.0)
        ones_bf = cpool.tile([P, 1], bf16, tag="onesb")
        nc.gpsimd.memset(ones_bf[:], 1.0)

        # weights arrive pre-packed partition-major: W[p, t*H+h].
        # wk rides first on sync (the first projection needs it); wv/wq
        # go behind chunk 0's scalar half, in first-use order.
        w_sb = {}
        for name, w in (("wk", Wk), ("wv", Wv), ("wq", Wq)):
            w_sb[name] = cpool.tile([P, DT * H], bf16, tag=name, name=name)


        kT = big.tile([P, N], bf16, tag="kT")          # K^T[h, n]
        qT = big.tile([P, NQ], bf16, tag="qT")         # Q^T[h, q]
        v_sb = big.tile([P, NT * H], bf16, tag="vsb")  # V[k%128, (kt, h)]
        # X^T resident in SBUF, chunk-major: xt[p, ((c, t), nb)]
        xt = big.tile([P, DT * N], bf16, tag="xt")
        xt4 = xt[:].rearrange("p (c t nb) -> p c t nb", c=NC, t=DT)

        # one contiguous 8KB-per-partition load per key chunk; chunk 0
        # split by partition halves across both HWDGE queues to land
        # first, later chunks alternate queues for aggregate bandwidth
        nc.sync.dma_start(xt4[0:64, 0], XT[0][0:64, :])
        nc.scalar.dma_start(xt4[64:128, 0], XT[0][64:128, :])
        # wk halves ride both fast queues right BEHIND chunk 0 (chunk-0
        # timing unchanged, wk lands ~3.5us earlier than the Pool queue)
        nc.sync.dma_start(w_sb["wk"][0:64, :], Wk[0:64, :])
        nc.scalar.dma_start(w_sb["wk"][64:128, :], Wk[64:128, :])
        nc.scalar.dma_start(w_sb["wv"][:], Wv)
        nc.scalar.dma_start(w_sb["wq"][:], Wq)
        # chunks 3 and 7 (most slack) ride the otherwise-idle Pool
        # SWDGE queue; the rest alternate the two fast HWDGE queues
        qmap = {1: nc.sync, 2: nc.scalar, 3: nc.gpsimd, 4: nc.sync,
                5: nc.scalar, 6: nc.sync, 7: nc.gpsimd}
        for c in range(1, NC):
            qmap[c].dma_start(xt4[:, c], XT[c])

        v_sb3 = v_sb[:].rearrange("p (kt h) -> p kt h", h=H)
        vxpose_todo = []  # (chunk, vT tile) awaiting transpose dispatch

        def flush_vxpose():
            while vxpose_todo:
                pc, vt = vxpose_todo.pop()
                nc.scalar.dma_start_transpose(
                    v_sb3[:, pc * NTC:(pc + 1) * NTC, :], vt[:]
                )

        def project(pp, wname, dst, c):
            """One projection for chunk c: 8 matmuls + one copy.

            K/Q copies go to DVE, V copies to ACT (keeps the vt ->
            vxpose ordering natural on the ACT queue)."""
            ps = pp.tile([P, CR], f32, tag="pps", name="pps")
            for t in range(DT):
                nc.tensor.matmul(
                    ps[:],
                    w_sb[wname][:, t * H:(t + 1) * H],
                    xt4[:, c, t],
                    start=(t == 0),
                    stop=(t == DT - 1),
                )
            if wname == "wv":
                nc.scalar.copy(dst, ps[:])
            else:
                nc.vector.tensor_copy(dst, ps[:])

        def proj_block(pp, c):
            flush_vxpose()
            project(pp, "wk", kT[:, c * CR:(c + 1) * CR], c)
            vt = vtp.tile([P, CR], bf16, tag="vt", name="vt")
            project(pp, "wv", vt[:], c)
            vxpose_todo.append((c, vt))
            if c * CR < NQ:
                project(pp, "wq", qT[:, c * CR:(c + 1) * CR], c)

        # ---- attention state ----
        acco = [None] * QCH    # PSUM accumulator tiles, one per q-chunk
        acc_d = [None] * QCH   # f32 chain heads (updated every 4th kt)
        tree = {}              # stashed pT / pair tiles for the bf16 tree
        pending = [None]       # (q_idx, kt, pT) with O-matmul not yet emitted

        def emit_O(q_idx, kt, pT):
            out_ps = acco[q_idx]
            for j in range(0, QC, FB):
                nc.tensor.matmul(
                    out_ps[:, j:j + FB],
                    v_sb[:, kt * H:(kt + 1) * H],
                    pT[:, j:j + FB],
                    start=(kt == 0), stop=(kt == NT - 1),
                )

        def attn_iter(q_idx, kt):
            q0 = q_idx * QC
            st = stp.tile([P, QC], f32, tag="st", name="st")
            for j in range(0, QC, FB):
                nc.tensor.matmul(
                    st[:, j:j + FB],
                    kT[:, kt * P:(kt + 1) * P],
                    qT[:, q0 + j:q0 + j + FB],
                    start=True, stop=True,
                )
            pT = ptp.tile([P, QC], bf16, tag="pt", name="pT")
            nc.scalar.activation(pT[:], st[:], AF.Exp, scale=SCALE)
            if pending[0] is not None:
                emit_O(*pending[0])
            pending[0] = (q_idx, kt, pT)
            if kt % 2 == 0:
                tree["pT"] = pT
            else:
                pair = accd.tile([P, QC], bf16, tag="pair", name="pair")
                nc.vector.tensor_add(pair[:], tree.pop("pT")[:], pT[:])
                if kt % 4 == 1:
                    tree["pair"] = pair
                else:
                    quad = accd.tile([P, QC], bf16, tag="quad", name="quad")
                    nc.vector.tensor_add(quad[:], tree.pop("pair")[:], pair[:])
                    if kt == NT - 1:
                        # last quad goes straight into the l-matmul (PE)
                        # instead of the serial f32 chain on the tail
                        tree[("lq", q_idx)] = quad
                    else:
                        nacc = accd.tile(
                            [P, QC], f32, tag=f"acc{q_idx}", name="nacc"
                        )
                        if kt == 3:
                            nc.vector.tensor_copy(nacc[:], quad[:])
                        else:
                            nc.vector.tensor_add(
                                nacc[:], acc_d[q_idx][:], quad[:]
                            )
                        acc_d[q_idx] = nacc

        # Epilogue, staged so each PE op's DVE input is long since ready.
        epi = {}

        def epi_lmm(q_idx):
            l_ps = stp.tile([P, QC], f32, tag="st", name="lps")  # row 0 used
            acc_m = acc_d[q_idx]            # covers kt 0..27
            lq = tree.pop(("lq", q_idx))    # covers kt 28..31
            for j in range(0, QC, FB):
                nc.tensor.matmul(
                    l_ps[0:1, j:j + FB], ones_col[:], acc_m[:, j:j + FB],
                    start=True, stop=False,
                )
                nc.tensor.matmul(
                    l_ps[0:1, j:j + FB], ones_bf[:], lq[:, j:j + FB],
                    start=False, stop=True,
                )
            epi[(q_idx, "l")] = l_ps

        def epi_recip(q_idx):
            r_sb = epp.tile([1, QC], f32, tag="rsb", name="rsb")
            for j in range(0, QC, FB):
                nc.vector.reciprocal_approx_fast(
                    out=r_sb[:, j:j + FB],
                    in_=epi[(q_idx, "l")][0:1, j:j + FB],
                )
            epi[(q_idx, "r")] = r_sb

        def epi_rbmm(q_idx):
            rb_ps = stp.tile([P, QC], f32, tag="st", name="rbps")
            for j in range(0, QC, FB):
                nc.tensor.matmul(
                    rb_ps[:, j:j + FB], ones_row[:],
                    epi[(q_idx, "r")][:, j:j + FB],
                    start=True, stop=True,
                )
            epi[(q_idx, "rb")] = rb_ps

        def epi_out(q_idx):
            # at the q1 tail the ACT engine is idle: let it do the PSUM
            # copy so the DVE only runs the multiplies
            q0 = q_idx * QC
            rb_sb = epp.tile([P, QC], f32, tag="rbsb", name="rbsb")
            o_sb = epp.tile([P, QC], f32, tag="osb", name="osb")
            for j in range(0, QC, FB):
                if q_idx == 0:
                    nc.vector.tensor_copy(
                        rb_sb[:, j:j + FB], epi[(q_idx, "rb")][:, j:j + FB]
                    )
                else:
                    nc.scalar.copy(
                        rb_sb[:, j:j + FB], epi[(q_idx, "rb")][:, j:j + FB]
                    )
                nc.vector.tensor_mul(
                    o_sb[:, j:j + FB],
                    acco[q_idx][:, j:j + FB], rb_sb[:, j:j + FB],
                )
                nc.sync.dma_start(
                    O[:, q0 + j:q0 + j + FB], o_sb[:, j:j + FB]
                )

        # ---- interleaved emission ----
        with tc.tile_pool(name="acc0", bufs=1, space="PSUM") as a0:
            acco[0] = a0.tile([P, QC], f32, tag="out0", name="out0")
            with tc.tile_pool(name="pp", bufs=2, space="PSUM") as pp:
                proj_block(pp, 0)
                proj_block(pp, 1)
                for c in range(2, NC):
                    proj_block(pp, c)
                    for kt in range(NTC * (c - 2), NTC * (c - 1)):
                        attn_iter(0, kt)
            flush_vxpose()
            # phase B head: finish q-chunk 0 (keys from chunks 6-7)
            for kt in range(NTC * (NC - 2), NT):
                attn_iter(0, kt)
            # pp closed above -> banks free for q-chunk 1's accumulator
            with tc.tile_pool(name="acc1", bufs=1, space="PSUM") as a1:
                acco[1] = a1.tile([P, QC], f32, tag="out1", name="out1")
                for kt in range(NT):
                    attn_iter(1, kt)
                    if kt == 4:
                        epi_lmm(0)
                    elif kt == 6:
                        epi_recip(0)
                    elif kt == 8:
                        epi_rbmm(0)
                    elif kt == 10:
                        epi_out(0)
                emit_O(*pending[0])
                pending[0] = None
                epi_lmm(1)
                epi_recip(1)
                epi_rbmm(1)
                epi_out(1)


def build_bass():
    import concourse.mybir as mybir
    from concourse import bacc
    from concourse.tile import TileContext

    dt = mybir.dt
    nc = bacc.Bacc("TRN2", target_bir_lowering=False, debug=False)
    XT = nc.dram_tensor(
        "XT", [N // CR, P, (D // P) * CR], dt.bfloat16, kind="ExternalInput"
    ).ap()
    Wq = nc.dram_tensor(
        "Wq", [P, (D // P) * H], dt.bfloat16, kind="ExternalInput"
    ).ap()
    Wk = nc.dram_tensor(
        "Wk", [P, (D // P) * H], dt.bfloat16, kind="ExternalInput"
    ).ap()
    Wv = nc.dram_tensor(
        "Wv", [P, (D // P) * H], dt.bfloat16, kind="ExternalInput"
    ).ap()
    O = nc.dram_tensor("O", [H, NQ], dt.float32, kind="ExternalOutput").ap()

    with TileContext(nc) as tc:
        emit_attention(tc, XT, Wq, Wk, Wv, O)
    nc.compile()
    return nc


_CACHED = {}


def _get_nc():
    if "nc" not in _CACHED:
        _CACHED["nc"] = build_bass()
    return _CACHED["nc"]


def kernel(X, Wq, Wk, Wv, trace=False):
    """Full-input entry point: X [4,4096,1024] f32 -> [4,4096,128] f32."""
    from concourse.bass_utils import run_bass_kernel_spmd

    X = np.ascontiguousarray(X, dtype=np.float32)

    def pack_w(w):
        # W[d, h] -> W[p, t*H+h] with d = t*P + p (partition-major)
        w = np.asarray(w, dtype=np.float32).astype(BF16)
        return np.ascontiguousarray(
            w.reshape(D // P, P, H).transpose(1, 0, 2).reshape(P, -1)
        )

    wq, wk, wv = pack_w(Wq), pack_w(Wk), pack_w(Wv)

    nc = _get_nc()
    in_maps = []
    for core in range(NCORES):
        b, half = core // QSPLIT, core % QSPLIT
        # X^T (bf16) packed partition-major per key chunk, rolled so
        # this core's queries are columns [0:NQ) of X^T
        xb = X[b]
        if half:
            xb = np.concatenate([xb[NQ:], xb[:NQ]], axis=0)
        # XTB[c, p, t, nb] = X[c*CR+nb, t*P+p]
        xtb = np.ascontiguousarray(
            xb.reshape(N // CR, CR, D // P, P).transpose(0, 3, 2, 1)
            .astype(BF16)
            .reshape(N // CR, P, (D // P) * CR)
        )
        in_maps.append({"XT": xtb, "Wq": wq, "Wk": wk, "Wv": wv})

    res = run_bass_kernel_spmd(
        nc, in_maps, core_ids=list(range(NCORES)), trace=trace
    )
    out = np.empty((B, N, H), dtype=np.float32)
    for core in range(NCORES):
        b, half = core // QSPLIT, core % QSPLIT
        out[b, half * NQ:(half + 1) * NQ] = res.results[core]["O"].T
    if trace:
        return out, res
    return out


# revision 40
# speedup vs baseline: 1.0319x; 1.0049x over previous
"""Trainium2 Bass kernel for a single attention head.

Problem: X[4,4096,1024], Wq/Wk/Wv[1024,128] ->
  softmax((X@Wq)(X@Wk)^T / sqrt(1024)) @ (X@Wv)   -> [4,4096,128]

Sharding: 8 cores = 4 batches x 2 query-halves. The host hands each core
X^T (bf16, [1024, 4096]) of its batch, rolled so the core's query half is
columns [0:2048) — a pure layout/dtype transform; all FLOPs (projections,
scores, softmax, output) run on device.

On-core pipeline (matmuls bf16 in, fp32 PSUM):
  - X^T is loaded with plain contiguous HWDGE DMAs, column-chunk-major so
    projections start after the first chunk (~3us).
  - Projections per 512-key chunk: K^T[h,n], V^T[h,k-chunk] (-> V[k,h] via
    a small XBAR transpose), Q^T[h,q] for chunks 0-3. PSUM->SBUF copies on
    the ACT engine.
  - Transposed flash attention, software-pipelined so the PE never stalls
    on the exp: emit S(kt), exp(kt), O(kt-1). Projection chunks are
    interleaved into the attention stream to keep the PE dense.
  - Softmax denominator: one DVE chain accumulating pT; l via ones-matmul;
    1/l via reciprocal_approx_fast; broadcast to [128,q] by a rank-1 ones
    matmul; one DVE multiply. The q0 epilogue is staged across q1
    iterations so the PE queue never blocks on DVE results.
  - O^T [h, q] is DMA'd out and transposed on the host.
"""

import numpy as np
import ml_dtypes

BF16 = np.dtype(ml_dtypes.bfloat16)

B, N, D, H = 4, 4096, 1024, 128
NCORES = 8
QSPLIT = 2  # cores per batch (query halves)
NQ = N // QSPLIT
SCALE = 1.0 / float(np.sqrt(np.float32(D)))
P = 128   # partitions
FB = 512  # matmul free-dim block (one fp32 PSUM bank)
CR = 512  # keys per projection chunk
QC = 1024  # query chunk (attention)


def emit_attention(tc, XT, Wq, Wk, Wv, O):
    """Emit the single-core attention program into TileContext tc.

    XT: [NC, P, DT*CR] bf16 DRAM — X^T packed partition-major per key
    chunk: XT[c, p, t*CR+nb] = X[c*CR+nb, t*P+p] (queries are columns
    [0:NQ) of X^T). W*: [D, H] bf16; O: [H, NQ] f32 (transposed output).
    """
    import concourse.mybir as mybir

    nc = tc.nc
    dt = mybir.dt
    f32, bf16 = dt.float32, dt.bfloat16
    AF = mybir.ActivationFunctionType

    DT = D // P        # 8 contraction tiles for projections
    NT = N // P        # 32 key tiles
    NC = N // CR       # 8 key chunks
    NTC = CR // P      # 4 key tiles per chunk
    QCH = NQ // QC     # 2 query chunks

    from contextlib import ExitStack

    with ExitStack() as ctx:
        cpool = ctx.enter_context(tc.tile_pool(name="const", bufs=1))
        big = ctx.enter_context(tc.tile_pool(name="big", bufs=1))
        vtp = ctx.enter_context(tc.tile_pool(name="vt", bufs=4))
        ptp = ctx.enter_context(tc.tile_pool(name="pt", bufs=6))
        accd = ctx.enter_context(tc.tile_pool(name="accd", bufs=2))
        epp = ctx.enter_context(tc.tile_pool(name="ep", bufs=2))
        # PSUM: pp(2 banks) + stp(4) + acco q0(2) = 8; pp closes before
        # the q1 acco pool opens (phase B: stp 4 + acco0 2 + acco1 2).
        stp = ctx.enter_context(tc.tile_pool(name="stps", bufs=2, space="PSUM"))

        ones_col = cpool.tile([P, 1], f32, tag="onesc")
        nc.gpsimd.memset(ones_col[:], 1.0)
        ones_row = cpool.tile([1, P], f32, tag="onesr")
        nc.gpsimd.memset(ones_row[:], 1.0)
        ones_bf = cpool.tile([P, 1], bf16, tag="onesb")
        nc.gpsimd.memset(ones_bf[:], 1.0)

        # weights arrive pre-packed partition-major: W[p, t*H+h].
        # wk rides first on sync (the first projection needs it); wv/wq
        # go behind chunk 0's scalar half, in first-use order.
        w_sb = {}
        for name, w in (("wk", Wk), ("wv", Wv), ("wq", Wq)):
            w_sb[name] = cpool.tile([P, DT * H], bf16, tag=name, name=name)


        kT = big.tile([P, N], bf16, tag="kT")          # K^T[h, n]
        qT = big.tile([P, NQ], bf16, tag="qT")         # Q^T[h, q]
        v_sb = big.tile([P, NT * H], bf16, tag="vsb")  # V[k%128, (kt, h)]
        # X^T resident in SBUF, chunk-major: xt[p, ((c, t), nb)]
        xt = big.tile([P, DT * N], bf16, tag="xt")
        xt4 = xt[:].rearrange("p (c t nb) -> p c t nb", c=NC, t=DT)

        # one contiguous 8KB-per-partition load per key chunk; chunk 0
        # split by partition halves across both HWDGE queues to land
        # first, later chunks alternate queues for aggregate bandwidth
        nc.sync.dma_start(xt4[0:64, 0], XT[0][0:64, :])
        nc.scalar.dma_start(xt4[64:128, 0], XT[0][64:128, :])
        # wk halves ride both fast queues right BEHIND chunk 0 (chunk-0
        # timing unchanged, wk lands ~3.5us earlier than the Pool queue)
        nc.sync.dma_start(w_sb["wk"][0:64, :], Wk[0:64, :])
        nc.scalar.dma_start(w_sb["wk"][64:128, :], Wk[64:128, :])
        nc.scalar.dma_start(w_sb["wv"][:], Wv)
        nc.scalar.dma_start(w_sb["wq"][:], Wq)
        # chunks 3 and 7 (most slack) ride the otherwise-idle Pool
        # SWDGE queue; the rest alternate the two fast HWDGE queues
        qmap = {1: nc.sync, 2: nc.scalar, 3: nc.gpsimd, 4: nc.sync,
                5: nc.scalar, 6: nc.sync, 7: nc.gpsimd}
        for c in range(1, NC):
            qmap[c].dma_start(xt4[:, c], XT[c])

        v_sb3 = v_sb[:].rearrange("p (kt h) -> p kt h", h=H)
        vxpose_todo = []  # (chunk, vT tile) awaiting transpose dispatch

        def flush_vxpose():
            while vxpose_todo:
                pc, vt = vxpose_todo.pop()
                nc.scalar.dma_start_transpose(
                    v_sb3[:, pc * NTC:(pc + 1) * NTC, :], vt[:]
                )

        def project(pp, wname, dst, c):
            """One projection for chunk c: 8 matmuls + one copy.

            K/Q copies go to DVE, V copies to ACT (keeps the vt ->
            vxpose ordering natural on the ACT queue)."""
            ps = pp.tile([P, CR], f32, tag="pps", name="pps")
            for t in range(DT):
                nc.tensor.matmul(
                    ps[:],
                    w_sb[wname][:, t * H:(t + 1) * H],
                    xt4[:, c, t],
                    start=(t == 0),
                    stop=(t == DT - 1),
                )
            if wname == "wv":
                nc.scalar.copy(dst, ps[:])
            else:
                nc.vector.tensor_copy(dst, ps[:])

        def proj_block(pp, c):
            flush_vxpose()
            project(pp, "wk", kT[:, c * CR:(c + 1) * CR], c)
            vt = vtp.tile([P, CR], bf16, tag="vt", name="vt")
            project(pp, "wv", vt[:], c)
            vxpose_todo.append((c, vt))
            if c * CR < NQ:
                project(pp, "wq", qT[:, c * CR:(c + 1) * CR], c)

        # ---- attention state ----
        acco = [None] * QCH    # PSUM accumulator tiles, one per q-chunk
        acc_d = [None] * QCH   # f32 chain heads (updated every 4th kt)
        tree = {}              # stashed pT / pair tiles for the bf16 tree
        pending = [None]       # (q_idx, kt, pT) with O-matmul not yet emitted

        def emit_O(q_idx, kt, pT):
            out_ps = acco[q_idx]
            for j in range(0, QC, FB):
                nc.tensor.matmul(
                    out_ps[:, j:j + FB],
                    v_sb[:, kt * H:(kt + 1) * H],
                    pT[:, j:j + FB],
                    start=(kt == 0), stop=(kt == NT - 1),
                )

        def attn_iter(q_idx, kt):
            q0 = q_idx * QC
            st = stp.tile([P, QC], f32, tag="st", name="st")
            for j in range(0, QC, FB):
                nc.tensor.matmul(
                    st[:, j:j + FB],
                    kT[:, kt * P:(kt + 1) * P],
                    qT[:, q0 + j:q0 + j + FB],
                    start=True, stop=True,
                )
            pT = ptp.tile([P, QC], bf16, tag="pt", name="pT")
            nc.scalar.activation(pT[:], st[:], AF.Exp, scale=SCALE)
            if pending[0] is not None:
                emit_O(*pending[0])
            pending[0] = (q_idx, kt, pT)
            if kt % 2 == 0:
                tree["pT"] = pT
            else:
                pair = accd.tile([P, QC], bf16, tag="pair", name="pair")
                nc.vector.tensor_add(pair[:], tree.pop("pT")[:], pT[:])
                if kt % 4 == 1:
                    tree["pair"] = pair
                else:
                    quad = accd.tile([P, QC], bf16, tag="quad", name="quad")
                    nc.vector.tensor_add(quad[:], tree.pop("pair")[:], pair[:])
                    if kt == NT - 1:
                        # last quad goes straight into the l-matmul (PE)
                        # instead of the serial f32 chain on the tail
                        tree[("lq", q_idx)] = quad
                    else:
                        nacc = accd.tile(
                            [P, QC], f32, tag=f"acc{q_idx}", name="nacc"
                        )
                        if kt == 3:
                            nc.vector.tensor_copy(nacc[:], quad[:])
                        else:
                            nc.vector.tensor_add(
                                nacc[:], acc_d[q_idx][:], quad[:]
                            )
                        acc_d[q_idx] = nacc

        # Epilogue, staged so each PE op's DVE input is long since ready.
        epi = {}

        def epi_lmm(q_idx):
            l_ps = stp.tile([P, QC], f32, tag="st", name="lps")  # row 0 used
            acc_m = acc_d[q_idx]            # covers kt 0..27
            lq = tree.pop(("lq", q_idx))    # covers kt 28..31
            for j in range(0, QC, FB):
                nc.tensor.matmul(
                    l_ps[0:1, j:j + FB], ones_col[:], acc_m[:, j:j + FB],
                    start=True, stop=False,
                )
                nc.tensor.matmul(
                    l_ps[0:1, j:j + FB], ones_bf[:], lq[:, j:j + FB],
                    start=False, stop=True,
                )
            epi[(q_idx, "l")] = l_ps

        def epi_recip(q_idx):
            r_sb = epp.tile([1, QC], f32, tag="rsb", name="rsb")
            for j in range(0, QC, FB):
                nc.vector.reciprocal_approx_fast(
                    out=r_sb[:, j:j + FB],
                    in_=epi[(q_idx, "l")][0:1, j:j + FB],
                )
            epi[(q_idx, "r")] = r_sb

        def epi_rbmm(q_idx):
            rb_ps = stp.tile([P, QC], f32, tag="st", name="rbps")
            for j in range(0, QC, FB):
                nc.tensor.matmul(
                    rb_ps[:, j:j + FB], ones_row[:],
                    epi[(q_idx, "r")][:, j:j + FB],
                    start=True, stop=True,
                )
            epi[(q_idx, "rb")] = rb_ps

        def epi_out(q_idx):
            # at the q1 tail the ACT engine is idle: let it do the PSUM
            # copy so the DVE only runs the multiplies
            q0 = q_idx * QC
            rb_sb = epp.tile([P, QC], f32, tag="rbsb", name="rbsb")
            o_sb = epp.tile([P, QC], f32, tag="osb", name="osb")
            for j in range(0, QC, FB):
                if q_idx == 0:
                    nc.vector.tensor_copy(
                        rb_sb[:, j:j + FB], epi[(q_idx, "rb")][:, j:j + FB]
                    )
                else:
                    nc.scalar.copy(
                        rb_sb[:, j:j + FB], epi[(q_idx, "rb")][:, j:j + FB]
                    )
                nc.vector.tensor_mul(
                    o_sb[:, j:j + FB],
                    acco[q_idx][:, j:j + FB], rb_sb[:, j:j + FB],
                )
                # kernel end = last output-DMA completion: at the q1
                # tail, put the second half on the idle scalar queue so
                # the two 256KB halves drain in parallel
                eng = nc.scalar if (q_idx == 1 and j > 0) else nc.sync
                eng.dma_start(
                    O[:, q0 + j:q0 + j + FB], o_sb[:, j:j + FB]
                )

        # ---- interleaved emission ----
        with tc.tile_pool(name="acc0", bufs=1, space="PSUM") as a0:
            acco[0] = a0.tile([P, QC], f32, tag="out0", name="out0")
            with tc.tile_pool(name="pp", bufs=2, space="PSUM") as pp:
                proj_block(pp, 0)
                proj_block(pp, 1)
                for c in range(2, NC):
                    proj_block(pp, c)
                    for kt in range(NTC * (c - 2), NTC * (c - 1)):
                        attn_iter(0, kt)
            flush_vxpose()
            # phase B head: finish q-chunk 0 (keys from chunks 6-7)
            for kt in range(NTC * (NC - 2), NT):
                attn_iter(0, kt)
            # pp closed above -> banks free for q-chunk 1's accumulator
            with tc.tile_pool(name="acc1", bufs=1, space="PSUM") as a1:
                acco[1] = a1.tile([P, QC], f32, tag="out1", name="out1")
                for kt in range(NT):
                    attn_iter(1, kt)
                    if kt == 4:
                        epi_lmm(0)
                    elif kt == 6:
                        epi_recip(0)
                    elif kt == 8:
                        epi_rbmm(0)
                    elif kt == 10:
                        epi_out(0)
                emit_O(*pending[0])
                pending[0] = None
                epi_lmm(1)
                epi_recip(1)
                epi_rbmm(1)
                epi_out(1)


def build_bass():
    import concourse.mybir as mybir
    from concourse import bacc
    from concourse.tile import TileContext

    dt = mybir.dt
    nc = bacc.Bacc("TRN2", target_bir_lowering=False, debug=False)
    XT = nc.dram_tensor(
        "XT", [N // CR, P, (D // P) * CR], dt.bfloat16, kind="ExternalInput"
    ).ap()
    Wq = nc.dram_tensor(
        "Wq", [P, (D // P) * H], dt.bfloat16, kind="ExternalInput"
    ).ap()
    Wk = nc.dram_tensor(
        "Wk", [P, (D // P) * H], dt.bfloat16, kind="ExternalInput"
    ).ap()
    Wv = nc.dram_tensor(
        "Wv", [P, (D // P) * H], dt.bfloat16, kind="ExternalInput"
    ).ap()
    O = nc.dram_tensor("O", [H, NQ], dt.float32, kind="ExternalOutput").ap()

    with TileContext(nc) as tc:
        emit_attention(tc, XT, Wq, Wk, Wv, O)
    nc.compile()
    return nc


_CACHED = {}


def _get_nc():
    if "nc" not in _CACHED:
        _CACHED["nc"] = build_bass()
    return _CACHED["nc"]


def kernel(X, Wq, Wk, Wv, trace=False):
    """Full-input entry point: X [4,4096,1024] f32 -> [4,4096,128] f32."""
    from concourse.bass_utils import run_bass_kernel_spmd

    X = np.ascontiguousarray(X, dtype=np.float32)

    def pack_w(w):
        # W[d, h] -> W[p, t*H+h] with d = t*P + p (partition-major)
        w = np.asarray(w, dtype=np.float32).astype(BF16)
        return np.ascontiguousarray(
            w.reshape(D // P, P, H).transpose(1, 0, 2).reshape(P, -1)
        )

    wq, wk, wv = pack_w(Wq), pack_w(Wk), pack_w(Wv)

    nc = _get_nc()
    in_maps = []
    for core in range(NCORES):
        b, half = core // QSPLIT, core % QSPLIT
        # X^T (bf16) packed partition-major per key chunk, rolled so
        # this core's queries are columns [0:NQ) of X^T
        xb = X[b]
        if half:
            xb = np.concatenate([xb[NQ:], xb[:NQ]], axis=0)
        # XTB[c, p, t, nb] = X[c*CR+nb, t*P+p]
        xtb = np.ascontiguousarray(
            xb.reshape(N // CR, CR, D // P, P).transpose(0, 3, 2, 1)
            .astype(BF16)
            .reshape(N // CR, P, (D // P) * CR)
        )
        in_maps.append({"XT": xtb, "Wq": wq, "Wk": wk, "Wv": wv})

    res = run_bass_kernel_spmd(
        nc, in_maps, core_ids=list(range(NCORES)), trace=trace
    )
    out = np.empty((B, N, H), dtype=np.float32)
    for core in range(NCORES):
        b, half = core // QSPLIT, core % QSPLIT
        out[b, half * NQ:(half + 1) * NQ] = res.results[core]["O"].T
    if trace:
        return out, res
    return out
